# revision 14
# baseline (speedup 1.0000x reference)
"""Causal scaled-dot-product attention on 8 Trainium2 NeuronCores.

Problem: B=2, H=16, S=2048, D=64, fp32, causal mask.
Sharding: batch*heads (32) split 4-per-core across 8 cores; no collectives.

Per-core bass/Tile kernel (v3):

Phase 1 (pair-major 4-row blocks; head pair A/B stacked on SBUF
partitions 0-63 / 64-127, matmuls auto row-tiled 64x128 -> concurrent):
  - S^T[k, q] = (K^T)^T @ Q^T on PE fp16.
  - P^T layout per head: [0:2048) = the 16 diagonal 128x128 blocks
    packed contiguously; then the off-diagonal remainder of each row.
  - Diagonal exp: batched 4 rows -> one single-segment ScalarE table-exp
    per head (self-attention keys dominate softmax mass -> exact).
  - Off-diagonal exp: greedy load balance between ScalarE (exact) and
    DVE via one-op Schraudolph: int16(s*log2e*1024 + (15+c)*1024)
    bit-cast as fp16 == 2^(s*log2e) to ~3%; the sawtooth error cancels
    through softmax normalization (l sums the same approximations).
  - GPSIMD affine_select zeroes the diag upper triangle after exp.

Phase 2 (chain quads, lagging): for q-tile qt all 4 heads accumulate
O' and l into one PSUM bank ([128, 4, 65], V_aug=[V|1]); evacuated by
ScalarE-Copy/DVE-copy into SBUF and DMA'd out raw; host divides by l.
"""

import sys
import numpy as np
from contextlib import ExitStack

B, H, S, D = 2, 16, 2048, 64
N_CORES = 8
HEADS_PER_CORE = (B * H) // N_CORES  # 4
CH = 128             # k-chunk (partition tile)
PIECE_W = 512        # off-diag S^T piece width (1 PSUM bank per head)
DIAG_ROWS = 4        # rows per batched diagonal exp
SCALE = 1.0 / np.sqrt(D)
MM_DTYPE = "float16"
SCH_A = float(SCALE * np.log2(np.e) * 1024.0)
SCH_C = -0.058
SCH_B = float((15.0 + SCH_C) * 1024.0)

_NP_MM = {"float16": np.float16, "float32": np.float32}

for _p in ("/opt/trn_rl_repo", "/opt/pypackages"):
    if _p not in sys.path:
        sys.path.append(_p)


def _off_ro(ci, s_len, n_chunks):
    """packed offset of row ci's off-diagonal span (after diag region)."""
    # off-diag len of row j: (s_len - 128*j) - 128 = (s_len - CH) - CH*j - ...
    return n_chunks * CH + (s_len - CH) * ci - CH * (ci * (ci - 1)) // 2


def _build_program(n_heads, s_len, piece_w=PIECE_W, mm_dtype=MM_DTYPE):
    import concourse.bass as bass  # noqa: F401
    import concourse.bacc as bacc
    import concourse.tile as tile
    from concourse import mybir

    f32 = mybir.dt.float32
    i16 = mybir.dt.int16
    mmdt = getattr(mybir.dt, mm_dtype)
    n_chunks = s_len // CH
    n_pairs = (n_heads + 1) // 2
    DP1 = D + 1
    pt_len = _off_ro(n_chunks, s_len, n_chunks)

    nc = bacc.Bacc(
        "TRN2",
        target_bir_lowering=False,
        debug=False,
        num_devices=N_CORES,
    )

    qk_d = nc.dram_tensor("qk", [128, n_pairs, 2, s_len], mmdt, kind="ExternalInput").ap()
    v_d = nc.dram_tensor("v", [128, n_heads, n_chunks, DP1], mmdt, kind="ExternalInput").ap()
    o_d = nc.dram_tensor("o", [n_chunks // 2, DP1, n_heads * 2 * CH], f32, kind="ExternalOutput").ap()

    # static greedy engine balance (ns accumulated per engine)
    bal = {"sc": 0.0, "ve": 0.0}
    SC = lambda fd, nseg=1: (230.0 * nseg + fd) / 1.2
    VE = lambda fd: (120.0 + fd) / 0.96

    n_blocks = n_chunks // DIAG_ROWS

    def off_len(ci):
        return s_len - CH * ci - CH

    def blk_len(b):
        return DIAG_ROWS * CH + sum(off_len(4 * b + r) for r in range(DIAG_ROWS))

    def loc_diag(ci):
        return CH * (ci % DIAG_ROWS)

    def loc_off(ci):
        r = ci % DIAG_ROWS
        b = ci // DIAG_ROWS
        return DIAG_ROWS * CH + sum(off_len(4 * b + j) for j in range(r))

    with tile.TileContext(nc) as tc, ExitStack() as ctx:
        const = ctx.enter_context(tc.tile_pool(name="const", bufs=1))
        sb_pt = ctx.enter_context(tc.tile_pool(name="ptp", bufs=1))
        sb_st = ctx.enter_context(tc.tile_pool(name="stage", bufs=1))
        ps_s = ctx.enter_context(tc.tile_pool(name="pss", bufs=2, space="PSUM"))
        ps_d = ctx.enter_context(tc.tile_pool(name="psd", bufs=1, space="PSUM"))
        ps_o = ctx.enter_context(tc.tile_pool(name="pso", bufs=1, space="PSUM"))

        wu = const.tile([128, 512], mmdt)  # PE warmup scratch
        nc.gpsimd.memset(wu[:, 0:512], 0.0)
        kq = {p: const.tile([128, s_len], mmdt, name=f"kq{p}") for p in range(n_pairs)}
        qq = {p: const.tile([128, s_len], mmdt, name=f"qq{p}") for p in range(n_pairs)}
        v = const.tile([128, n_heads, n_chunks, DP1], mmdt)

        # PE warmup: ~3.5us of dummy matmuls so HAM un-throttles to 2.4GHz
        # while the input DMAs land.
        stw = ps_s.tile([128, 2, piece_w], f32, tag="st")
        for i in range(18):
            bp = 64 * (i % 2)
            nc.tensor.matmul(
                stw[:, i % 2, 0:piece_w],
                wu[bp:bp + 64, 0:CH],
                wu[bp:bp + 64, 0:piece_w],
                start=True, stop=True,
                tile_position=(bp, 0),
            )

        for pair in range(n_pairs):
            nc.sync.dma_start(out=kq[pair], in_=qk_d[:, pair, 1])
            nc.sync.dma_start(out=qq[pair], in_=qk_d[:, pair, 0])
        for hh in range(n_heads):
            nc.sync.dma_start(out=v[:, hh], in_=v_d[:, hh])

        pair_heads = {p: [hh for hh in (2 * p, 2 * p + 1) if hh < n_heads]
                      for p in range(n_pairs)}
        pts = {(p, b): sb_pt.tile([128, 2, blk_len(b)], mmdt, name=f"pt{p}_{b}")
               for p in range(n_pairs) for b in range(n_blocks)}
        stage = sb_st.tile([128, n_chunks // 2, n_heads * 2 * CH], f32, name="ostage")

        def exp_sc(pt_t, src_ap, idx0, nh, dst0, fd):
            nc.scalar.activation(
                pt_t[:, idx0:idx0 + nh, dst0:dst0 + fd],
                src_ap,
                mybir.ActivationFunctionType.Exp,
                scale=float(SCALE),
            )

        def exp_ve(pt_t, src_ap, idx0, nh, dst0, fd):
            nc.vector.tensor_scalar(
                pt_t[:, idx0:idx0 + nh, dst0:dst0 + fd].bitcast(i16),
                src_ap,
                SCH_A,
                SCH_B,
                mybir.AluOpType.mult,
                mybir.AluOpType.add,
            )

        def mm1_diag(pair, heads, ci, dt_):
            sp0 = CH * ci
            for idx, hh in enumerate(heads):
                bp = 64 * (hh % 2)
                nc.tensor.matmul(
                    dt_[:, idx, ci % DIAG_ROWS, :],
                    kq[pair][bp:bp + 64, sp0:sp0 + CH],
                    qq[pair][bp:bp + 64, sp0:sp0 + CH],
                    start=True, stop=True,
                    tile_position=(bp, 0),
                )

        def diag_finish(pair, heads, blk, dt_):
            pt_t = pts[(pair, blk)]
            for idx in range(len(heads)):
                bal["sc"] += SC(DIAG_ROWS * CH)
                exp_sc(pt_t, dt_[:, idx], idx, 1, 0, DIAG_ROWS * CH)
            for r in range(DIAG_ROWS):
                dg = CH * r
                for idx in range(len(heads)):
                    nc.gpsimd.affine_select(
                        out=pt_t[:, idx, dg:dg + CH],
                        in_=pt_t[:, idx, dg:dg + CH],
                        compare_op=mybir.AluOpType.is_ge,
                        fill=0.0,
                        base=0,
                        pattern=[[1, CH]],
                        channel_multiplier=-1,
                    )

        def off_row(pair, heads, ci):
            nh = len(heads)
            pt_t = pts[(pair, ci // DIAG_ROWS)]
            sp0 = CH * ci
            rem = off_len(ci)
            ro = loc_off(ci)
            poff = 0
            while poff < rem:
                w = min(piece_w, rem - poff)
                st = ps_s.tile([128, 2, piece_w], f32, tag="st")
                for idx, hh in enumerate(heads):
                    bp = 64 * (hh % 2)
                    nc.tensor.matmul(
                        st[:, idx, 0:w],
                        kq[pair][bp:bp + 64, sp0:sp0 + CH],
                        qq[pair][bp:bp + 64, sp0 + CH + poff:sp0 + CH + poff + w],
                        start=True, stop=True,
                        tile_position=(bp, 0),
                    )
                cb_ve = VE(2 * w)
                cb_sc = 2 * SC(w)
                c_sp = (SC(w), VE(w))
                opts = [
                    ("ve", max(bal["sc"], bal["ve"] + cb_ve)),
                    ("sc", max(bal["sc"] + cb_sc, bal["ve"])),
                    ("sp", max(bal["sc"] + c_sp[0], bal["ve"] + c_sp[1])),
                ]
                choice = min(opts, key=lambda o: o[1])[0]
                if choice == "ve":
                    bal["ve"] += cb_ve
                    exp_ve(pt_t, st[:, 0:nh, 0:w], 0, nh, ro + poff, w)
                elif choice == "sc":
                    bal["sc"] += cb_sc
                    for idx in range(nh):
                        exp_sc(pt_t, st[:, idx, 0:w], idx, 1, ro + poff, w)
                else:
                    bal["sc"] += c_sp[0]
                    bal["ve"] += c_sp[1]
                    exp_sc(pt_t, st[:, 0, 0:w], 0, 1, ro + poff, w)
                    exp_ve(pt_t, st[:, 1, 0:w], 1, 1, ro + poff, w)
                poff += w

        def block_rows(pair, heads, blk, quad_cb):
            """rows 4*blk..4*blk+3; last row: diag mm + batch exp + affine
            emitted BEFORE its off-diag pieces, so chain quads depending on
            this block's diagonal unblock as early as possible."""
            dt_ = ps_d.tile([128, 2, DIAG_ROWS, CH], f32, tag="dt")
            r0 = DIAG_ROWS * blk
            for r in range(DIAG_ROWS):
                ci = r0 + r
                last = (r == DIAG_ROWS - 1)
                if last:
                    mm1_diag(pair, heads, ci, dt_)
                    diag_finish(pair, heads, blk, dt_)
                    off_row(pair, heads, ci)
                else:
                    off_row(pair, heads, ci)
                    mm1_diag(pair, heads, ci, dt_)
                quad_cb()

        def chain_group(g2):
            """O' for q-tiles {2g2, 2g2+1} of all heads: V_aug stationary
            (65-col LDW), pt moving up to 256 cols -> few, wide matmuls."""
            qt0 = 2 * g2
            W = 2 * CH
            op = ps_o.tile([128, n_heads, W], f32, tag="op")
            for pair in range(n_pairs):
                for idx, hh in enumerate(pair_heads[pair]):
                    va = v[:, hh, :, :]
                    for ci in range(qt0):
                        mv0 = loc_off(ci) + CH * qt0 - CH * (ci + 1)
                        pt_t = pts[(pair, ci // DIAG_ROWS)]
                        nc.tensor.matmul(
                            op[0:DP1, hh, 0:W],
                            va[:, ci, :],
                            pt_t[:, idx, mv0:mv0 + W],
                            start=(ci == 0), stop=False,
                            skip_group_check=True,
                        )
                    pt_t = pts[(pair, qt0 // DIAG_ROWS)]
                    dg = loc_diag(qt0)
                    nc.tensor.matmul(
                        op[0:DP1, hh, 0:CH],
                        va[:, qt0, :],
                        pt_t[:, idx, dg:dg + CH],
                        start=(qt0 == 0), stop=False,
                        skip_group_check=True,
                    )
                    of = loc_off(qt0)
                    nc.tensor.matmul(
                        op[0:DP1, hh, CH:W],
                        va[:, qt0, :],
                        pt_t[:, idx, of:of + CH],
                        start=(qt0 == 0), stop=False,
                        skip_group_check=True,
                    )
                    qe = qt0 + 1
                    pt_t = pts[(pair, qe // DIAG_ROWS)]
                    dg = loc_diag(qe)
                    nc.tensor.matmul(
                        op[0:DP1, hh, CH:W],
                        va[:, qe, :],
                        pt_t[:, idx, dg:dg + CH],
                        start=False, stop=True,
                        skip_group_check=True,
                    )
            fd = n_heads * W
            if bal["sc"] + SC(fd) <= bal["ve"] + VE(fd):
                bal["sc"] += SC(fd)
                nc.scalar.activation(
                    stage[0:DP1, g2], op[0:DP1, :, :],
                    mybir.ActivationFunctionType.Copy,
                )
            else:
                bal["ve"] += VE(fd)
                nc.vector.tensor_copy(stage[0:DP1, g2], op[0:DP1, :, :])
            nc.sync.dma_start(out=o_d[g2], in_=stage[0:DP1, g2])

        # groups lag ~a block behind phase 1
        ready = []
        emitted = [0]
        n_groups = n_chunks // 2

        def quad_cb():
            if ready:
                chain_group(ready.pop(0))

        for blk in range(n_blocks):
            for pair in range(n_pairs):
                block_rows(pair, pair_heads[pair], blk, quad_cb)
            hi_g = (DIAG_ROWS * blk + 1) // 2  # groups with both qts <= 4*blk
            while emitted[0] < min(hi_g, n_groups):
                ready.append(emitted[0])
                emitted[0] += 1
        while emitted[0] < n_groups:
            ready.append(emitted[0])
            emitted[0] += 1
        while ready:
            chain_group(ready.pop(0))

    nc.compile()
    return nc


_PROGRAM_CACHE = {}


def _get_program(n_heads=HEADS_PER_CORE, s_len=S, piece_w=PIECE_W, mm_dtype=MM_DTYPE):
    key = (n_heads, s_len, piece_w, mm_dtype)
    if key not in _PROGRAM_CACHE:
        _PROGRAM_CACHE[key] = _build_program(n_heads, s_len, piece_w, mm_dtype)
    return _PROGRAM_CACHE[key]


def _pack_core(Qf, Kf, Vf, heads, s_len=S, mm_dtype=MM_DTYPE):
    """Build the per-core input dict. Qf/Kf/Vf: [B*H, S, D] float32."""
    dt_np = _NP_MM[mm_dtype]
    n_heads = len(heads)
    n_pairs = (n_heads + 1) // 2
    n_chunks = s_len // CH
    qk = np.zeros((128, n_pairs, 2, s_len), dt_np)
    v = np.ones((128, n_heads, n_chunks, D + 1), dt_np)
    for i, hf in enumerate(heads):
        pair, side = divmod(i, 2)
        bp = 64 * side
        qk[bp:bp + 64, pair, 0] = Qf[hf].T
        qk[bp:bp + 64, pair, 1] = Kf[hf].T
        v[:, i, :, :D] = Vf[hf].reshape(n_chunks, CH, D).transpose(1, 0, 2)
    return {"qk": qk, "v": v}


def _unpack_core(o_np, s_len=S):
    """o_np: [n_chunks//2, D+1, n_heads*256] raw O'^T -> [n_heads, S, D]."""
    n_groups = s_len // CH // 2
    o = o_np.reshape(n_groups, D + 1, -1, 2 * CH)
    n_heads = o.shape[2]
    out = o[:, :D] / o[:, D:D + 1]          # [g2, d, h, j]
    return out.transpose(2, 0, 3, 1).reshape(n_heads, s_len, D).astype(np.float32)


def kernel(Q, K, V, mask):
    Q = np.asarray(Q, np.float32)
    K = np.asarray(K, np.float32)
    V = np.asarray(V, np.float32)
    mask = np.asarray(mask)

    if not np.array_equal(mask, np.tril(np.ones((S, S), dtype=bool))):
        scores = np.einsum("bhqd,bhkd->bhqk", Q, K) * SCALE
        scores = np.where(mask, scores, -np.inf)
        scores -= scores.max(-1, keepdims=True)
        p = np.exp(scores)
        p /= p.sum(-1, keepdims=True)
        return np.einsum("bhqk,bhkd->bhqd", p, V).astype(np.float32)

    from concourse.bass_utils import run_bass_kernel_spmd

    Qf = Q.reshape(B * H, S, D)
    Kf = K.reshape(B * H, S, D)
    Vf = V.reshape(B * H, S, D)

    nc = _get_program()
    in_maps = [
        _pack_core(Qf, Kf, Vf, list(range(c * HEADS_PER_CORE, (c + 1) * HEADS_PER_CORE)))
        for c in range(N_CORES)
    ]
    res = run_bass_kernel_spmd(nc, in_maps, core_ids=list(range(N_CORES)))
    out = np.empty((B * H, S, D), np.float32)
    for c in range(N_CORES):
        out[c * HEADS_PER_CORE:(c + 1) * HEADS_PER_CORE] = _unpack_core(res.results[c]["o"])
    return out.reshape(B, H, S, D)


# revision 15
# speedup vs baseline: 1.1610x; 1.1610x over previous
"""Causal scaled-dot-product attention on 8 Trainium2 NeuronCores.

Problem: B=2, H=16, S=2048, D=64, fp32, causal mask.
Sharding: batch*heads (32) split 4-per-core across 8 cores; no collectives.

Per-core bass/Tile kernel (v3):

Phase 1 (pair-major 4-row blocks; head pair A/B stacked on SBUF
partitions 0-63 / 64-127, matmuls auto row-tiled 64x128 -> concurrent):
  - S^T[k, q] = (K^T)^T @ Q^T on PE fp16.
  - P^T layout per head: [0:2048) = the 16 diagonal 128x128 blocks
    packed contiguously; then the off-diagonal remainder of each row.
  - Diagonal exp: batched 4 rows -> one single-segment ScalarE table-exp
    per head (self-attention keys dominate softmax mass -> exact).
  - Off-diagonal exp: greedy load balance between ScalarE (exact) and
    DVE via one-op Schraudolph: int16(s*log2e*1024 + (15+c)*1024)
    bit-cast as fp16 == 2^(s*log2e) to ~3%; the sawtooth error cancels
    through softmax normalization (l sums the same approximations).
  - GPSIMD affine_select zeroes the diag upper triangle after exp.

Phase 2 (chain quads, lagging): for q-tile qt all 4 heads accumulate
O' and l into one PSUM bank ([128, 4, 65], V_aug=[V|1]); evacuated by
ScalarE-Copy/DVE-copy into SBUF and DMA'd out raw; host divides by l.
"""

import sys
import numpy as np
from contextlib import ExitStack

B, H, S, D = 2, 16, 2048, 64
N_CORES = 8
HEADS_PER_CORE = (B * H) // N_CORES  # 4
CH = 128             # k-chunk (partition tile)
PIECE_W = 512        # off-diag S^T piece width (1 PSUM bank per head)
DIAG_ROWS = 4        # rows per batched diagonal exp
SCALE = 1.0 / np.sqrt(D)
MM_DTYPE = "float16"
SCH_A = float(SCALE * np.log2(np.e) * 1024.0)
SCH_C = -0.058
SCH_B = float((15.0 + SCH_C) * 1024.0)

_NP_MM = {"float16": np.float16, "float32": np.float32}

for _p in ("/opt/trn_rl_repo", "/opt/pypackages"):
    if _p not in sys.path:
        sys.path.append(_p)


def _off_ro(ci, s_len, n_chunks):
    """packed offset of row ci's off-diagonal span (after diag region)."""
    # off-diag len of row j: (s_len - 128*j) - 128 = (s_len - CH) - CH*j - ...
    return n_chunks * CH + (s_len - CH) * ci - CH * (ci * (ci - 1)) // 2


def _build_program(n_heads, s_len, piece_w=PIECE_W, mm_dtype=MM_DTYPE):
    import concourse.bass as bass  # noqa: F401
    import concourse.bacc as bacc
    import concourse.tile as tile
    from concourse import mybir

    f32 = mybir.dt.float32
    i16 = mybir.dt.int16
    mmdt = getattr(mybir.dt, mm_dtype)
    n_chunks = s_len // CH
    n_pairs = (n_heads + 1) // 2
    DP1 = D + 1
    pt_len = _off_ro(n_chunks, s_len, n_chunks)

    nc = bacc.Bacc(
        "TRN2",
        target_bir_lowering=False,
        debug=False,
        num_devices=N_CORES,
    )

    qk_d = nc.dram_tensor("qk", [128, n_pairs, 2, s_len], mmdt, kind="ExternalInput").ap()
    v_d = nc.dram_tensor("v", [128, n_heads, n_chunks, DP1], mmdt, kind="ExternalInput").ap()
    o_d = nc.dram_tensor("o", [n_chunks, 128, n_heads * DP1], f32, kind="ExternalOutput").ap()

    # static greedy engine balance (ns accumulated per engine)
    bal = {"sc": 0.0, "ve": 0.0}
    SC = lambda fd, nseg=1: (230.0 * nseg + fd) / 1.2
    VE = lambda fd: 2.0 * (120.0 + fd) / 0.96 - 266.0

    n_blocks = n_chunks // DIAG_ROWS

    def off_len(ci):
        return s_len - CH * ci - CH

    def blk_len(b):
        return DIAG_ROWS * CH + sum(off_len(4 * b + r) for r in range(DIAG_ROWS))

    def loc_diag(ci):
        return CH * (ci % DIAG_ROWS)

    def loc_off(ci):
        r = ci % DIAG_ROWS
        b = ci // DIAG_ROWS
        return DIAG_ROWS * CH + sum(off_len(4 * b + j) for j in range(r))

    with tile.TileContext(nc) as tc, ExitStack() as ctx:
        const = ctx.enter_context(tc.tile_pool(name="const", bufs=1))
        sb_pt = ctx.enter_context(tc.tile_pool(name="ptp", bufs=1))
        sb_st = ctx.enter_context(tc.tile_pool(name="stage", bufs=1))
        ps_s = ctx.enter_context(tc.tile_pool(name="pss", bufs=2, space="PSUM"))
        ps_d = ctx.enter_context(tc.tile_pool(name="psd", bufs=1, space="PSUM"))
        ps_o = ctx.enter_context(tc.tile_pool(name="pso", bufs=2, space="PSUM"))

        wu = const.tile([128, 512], mmdt)  # PE warmup scratch
        nc.gpsimd.memset(wu[:, 0:512], 0.0)
        kq = {p: const.tile([128, s_len], mmdt, name=f"kq{p}") for p in range(n_pairs)}
        qq = {p: const.tile([128, s_len], mmdt, name=f"qq{p}") for p in range(n_pairs)}
        v = const.tile([128, n_heads, n_chunks, DP1], mmdt)

        # PE warmup: ~3.5us of dummy matmuls so HAM un-throttles to 2.4GHz
        # while the input DMAs land.
        stw = ps_s.tile([128, 2, piece_w], f32, tag="st")
        for i in range(18):
            bp = 64 * (i % 2)
            nc.tensor.matmul(
                stw[:, i % 2, 0:piece_w],
                wu[bp:bp + 64, 0:CH],
                wu[bp:bp + 64, 0:piece_w],
                start=True, stop=True,
                tile_position=(bp, 0),
            )

        for pair in range(n_pairs):
            nc.sync.dma_start(out=kq[pair], in_=qk_d[:, pair, 1])
            nc.sync.dma_start(out=qq[pair], in_=qk_d[:, pair, 0])
        for hh in range(n_heads):
            nc.sync.dma_start(out=v[:, hh], in_=v_d[:, hh])

        pair_heads = {p: [hh for hh in (2 * p, 2 * p + 1) if hh < n_heads]
                      for p in range(n_pairs)}
        pts = {(p, b): sb_pt.tile([128, 2, blk_len(b)], mmdt, name=f"pt{p}_{b}")
               for p in range(n_pairs) for b in range(n_blocks)}
        stage = sb_st.tile([128, n_chunks, n_heads * DP1], f32, name="ostage")

        def exp_sc(pt_t, src_ap, idx0, nh, dst0, fd):
            nc.scalar.activation(
                pt_t[:, idx0:idx0 + nh, dst0:dst0 + fd],
                src_ap,
                mybir.ActivationFunctionType.Exp,
                scale=float(SCALE),
            )

        def exp_ve(pt_t, src_ap, idx0, nh, dst0, fd):
            nc.vector.tensor_scalar(
                pt_t[:, idx0:idx0 + nh, dst0:dst0 + fd].bitcast(i16),
                src_ap,
                SCH_A,
                SCH_B,
                mybir.AluOpType.mult,
                mybir.AluOpType.add,
            )

        def mm1_diag(pair, heads, ci, dt_):
            sp0 = CH * ci
            for idx, hh in enumerate(heads):
                bp = 64 * (hh % 2)
                nc.tensor.matmul(
                    dt_[:, idx, ci % DIAG_ROWS, :],
                    kq[pair][bp:bp + 64, sp0:sp0 + CH],
                    qq[pair][bp:bp + 64, sp0:sp0 + CH],
                    start=True, stop=True,
                    tile_position=(bp, 0),
                )

        def diag_finish(pair, heads, blk, dt_):
            pt_t = pts[(pair, blk)]
            for idx in range(len(heads)):
                bal["sc"] += SC(DIAG_ROWS * CH)
                exp_sc(pt_t, dt_[:, idx], idx, 1, 0, DIAG_ROWS * CH)
            for r in range(DIAG_ROWS):
                dg = CH * r
                for idx in range(len(heads)):
                    nc.gpsimd.affine_select(
                        out=pt_t[:, idx, dg:dg + CH],
                        in_=pt_t[:, idx, dg:dg + CH],
                        compare_op=mybir.AluOpType.is_ge,
                        fill=0.0,
                        base=0,
                        pattern=[[1, CH]],
                        channel_multiplier=-1,
                    )

        def off_row(pair, heads, ci):
            nh = len(heads)
            pt_t = pts[(pair, ci // DIAG_ROWS)]
            sp0 = CH * ci
            rem = off_len(ci)
            ro = loc_off(ci)
            poff = 0
            while poff < rem:
                w = min(piece_w, rem - poff)
                st = ps_s.tile([128, 2, piece_w], f32, tag="st")
                for idx, hh in enumerate(heads):
                    bp = 64 * (hh % 2)
                    nc.tensor.matmul(
                        st[:, idx, 0:w],
                        kq[pair][bp:bp + 64, sp0:sp0 + CH],
                        qq[pair][bp:bp + 64, sp0 + CH + poff:sp0 + CH + poff + w],
                        start=True, stop=True,
                        tile_position=(bp, 0),
                    )
                cb_ve = VE(2 * w)
                cb_sc = 2 * SC(w)
                c_sp = (SC(w), VE(w))
                opts = [
                    ("ve", max(bal["sc"], bal["ve"] + cb_ve)),
                    ("sc", max(bal["sc"] + cb_sc, bal["ve"])),
                    ("sp", max(bal["sc"] + c_sp[0], bal["ve"] + c_sp[1])),
                ]
                choice = min(opts, key=lambda o: o[1])[0]
                if choice == "ve":
                    bal["ve"] += cb_ve
                    exp_ve(pt_t, st[:, 0:nh, 0:w], 0, nh, ro + poff, w)
                elif choice == "sc":
                    bal["sc"] += cb_sc
                    for idx in range(nh):
                        exp_sc(pt_t, st[:, idx, 0:w], idx, 1, ro + poff, w)
                else:
                    bal["sc"] += c_sp[0]
                    bal["ve"] += c_sp[1]
                    exp_sc(pt_t, st[:, 0, 0:w], 0, 1, ro + poff, w)
                    exp_ve(pt_t, st[:, 1, 0:w], 1, 1, ro + poff, w)
                poff += w

        def block_rows(pair, heads, blk, quad_cb):
            """rows 4*blk..4*blk+3; last row: diag mm + batch exp + affine
            emitted BEFORE its off-diag pieces, so chain quads depending on
            this block's diagonal unblock as early as possible."""
            dt_ = ps_d.tile([128, 2, DIAG_ROWS, CH], f32, tag="dt")
            r0 = DIAG_ROWS * blk
            for r in range(DIAG_ROWS):
                ci = r0 + r
                last = (r == DIAG_ROWS - 1)
                if last:
                    mm1_diag(pair, heads, ci, dt_)
                    diag_finish(pair, heads, blk, dt_)
                    off_row(pair, heads, ci)
                else:
                    off_row(pair, heads, ci)
                    mm1_diag(pair, heads, ci, dt_)
                quad_cb()

        def chain_quad(qt):
            op = ps_o.tile([128, n_heads, DP1], f32, tag="op")
            for pair in range(n_pairs):
                for idx, hh in enumerate(pair_heads[pair]):
                    for ci in range(qt + 1):
                        pt_t = pts[(pair, ci // DIAG_ROWS)]
                        if ci == qt:
                            sl = loc_diag(ci)
                        else:
                            sl = loc_off(ci) + CH * (qt - ci - 1)
                        nc.tensor.matmul(
                            op[:, hh, :],
                            pt_t[:, idx, sl:sl + CH],
                            v[:, hh, ci, :],
                            start=(ci == 0),
                            stop=(ci == qt),
                        )
            fd = n_heads * DP1
            if bal["sc"] + SC(fd) <= bal["ve"] + VE(fd):
                bal["sc"] += SC(fd)
                nc.scalar.activation(
                    stage[:, qt], op[:, :, :],
                    mybir.ActivationFunctionType.Copy,
                )
            else:
                bal["ve"] += VE(fd)
                nc.vector.tensor_copy(stage[:, qt], op[:, :, :])
            nc.sync.dma_start(out=o_d[qt], in_=stage[:, qt])

        ready = []
        emitted = [0]

        def quad_cb():
            if ready:
                chain_quad(ready.pop(0))

        for blk in range(n_blocks):
            for pair in range(n_pairs):
                block_rows(pair, pair_heads[pair], blk, quad_cb)
            hi = DIAG_ROWS * blk + 1
            while emitted[0] < min(hi, n_chunks):
                ready.append(emitted[0])
                emitted[0] += 1
        while emitted[0] < n_chunks:
            ready.append(emitted[0])
            emitted[0] += 1
        while ready:
            chain_quad(ready.pop(0))

    nc.compile()
    return nc


_PROGRAM_CACHE = {}


def _get_program(n_heads=HEADS_PER_CORE, s_len=S, piece_w=PIECE_W, mm_dtype=MM_DTYPE):
    key = (n_heads, s_len, piece_w, mm_dtype)
    if key not in _PROGRAM_CACHE:
        _PROGRAM_CACHE[key] = _build_program(n_heads, s_len, piece_w, mm_dtype)
    return _PROGRAM_CACHE[key]


def _pack_core(Qf, Kf, Vf, heads, s_len=S, mm_dtype=MM_DTYPE):
    """Build the per-core input dict. Qf/Kf/Vf: [B*H, S, D] float32."""
    dt_np = _NP_MM[mm_dtype]
    n_heads = len(heads)
    n_pairs = (n_heads + 1) // 2
    n_chunks = s_len // CH
    qk = np.zeros((128, n_pairs, 2, s_len), dt_np)
    v = np.ones((128, n_heads, n_chunks, D + 1), dt_np)
    for i, hf in enumerate(heads):
        pair, side = divmod(i, 2)
        bp = 64 * side
        qk[bp:bp + 64, pair, 0] = Qf[hf].T
        qk[bp:bp + 64, pair, 1] = Kf[hf].T
        v[:, i, :, :D] = Vf[hf].reshape(n_chunks, CH, D).transpose(1, 0, 2)
    return {"qk": qk, "v": v}


def _unpack_core(o_np, s_len=S):
    """o_np: [n_chunks, 128, n_heads*(D+1)] raw -> [n_heads, S, D]."""
    n_chunks = s_len // CH
    o = o_np.reshape(n_chunks, 128, -1, D + 1)
    n_heads = o.shape[2]
    out = o[:, :, :, :D] / o[:, :, :, D:D + 1]
    return out.transpose(2, 0, 1, 3).reshape(n_heads, s_len, D).astype(np.float32)


def kernel(Q, K, V, mask):
    Q = np.asarray(Q, np.float32)
    K = np.asarray(K, np.float32)
    V = np.asarray(V, np.float32)
    mask = np.asarray(mask)

    if not np.array_equal(mask, np.tril(np.ones((S, S), dtype=bool))):
        scores = np.einsum("bhqd,bhkd->bhqk", Q, K) * SCALE
        scores = np.where(mask, scores, -np.inf)
        scores -= scores.max(-1, keepdims=True)
        p = np.exp(scores)
        p /= p.sum(-1, keepdims=True)
        return np.einsum("bhqk,bhkd->bhqd", p, V).astype(np.float32)

    from concourse.bass_utils import run_bass_kernel_spmd

    Qf = Q.reshape(B * H, S, D)
    Kf = K.reshape(B * H, S, D)
    Vf = V.reshape(B * H, S, D)

    nc = _get_program()
    in_maps = [
        _pack_core(Qf, Kf, Vf, list(range(c * HEADS_PER_CORE, (c + 1) * HEADS_PER_CORE)))
        for c in range(N_CORES)
    ]
    res = run_bass_kernel_spmd(nc, in_maps, core_ids=list(range(N_CORES)))
    out = np.empty((B * H, S, D), np.float32)
    for c in range(N_CORES):
        out[c * HEADS_PER_CORE:(c + 1) * HEADS_PER_CORE] = _unpack_core(res.results[c]["o"])
    return out.reshape(B, H, S, D)


# revision 17
# speedup vs baseline: 1.2579x; 1.0834x over previous
"""Causal scaled-dot-product attention on 8 Trainium2 NeuronCores.

Problem: B=2, H=16, S=2048, D=64, fp32, causal mask.
Sharding: batch*heads (32) split 4-per-core across 8 cores; no collectives.

Per-core bass/Tile kernel (v10):
  - mm1: S^T[k,q] = (K^T)^T @ Q^T, fp16, head pair A/B on SBUF partition
    halves -> 64x128 row-tiled matmuls run concurrently (T0/T8).
  - exp: per 512-wide piece from PSUM. The 128-wide diagonal block of
    each row is always exact ScalarE table-exp (self-attention keys
    dominate softmax mass); everything else is load-balanced between
    ScalarE (exact) and DVE via a one-op Schraudolph:
    int16(s*log2e*1024 + (15+c)*1024) bit-cast as fp16 ~= e^s to ~3%;
    the sawtooth cancels through softmax normalization. DVE cost model
    includes its post-op pipeline DRAIN (~dur-266ns).
  - GPSIMD affine_select zeroes the diag upper triangle after exp.
  - Phase 2 (lagging 2 rows): per q-tile all 4 heads accumulate O'|l
    into one PSUM bank ([128, 4, 65], V_aug = [V|1] moving, pt chunks
    stationary); evacuated raw to SBUF/DRAM; host divides by l.
  - PSUM: score pieces 3x[128,2,512] (6 banks) + O' accum 2x1 bank.
"""

import sys
import numpy as np
from contextlib import ExitStack

B, H, S, D = 2, 16, 2048, 64
N_CORES = 8
HEADS_PER_CORE = (B * H) // N_CORES  # 4
CH = 128             # k-chunk (partition tile)
PIECE_W = 512        # S^T piece width (1 PSUM bank per head)
SCALE = 1.0 / np.sqrt(D)
MM_DTYPE = "float16"
SCH_A = float(SCALE * np.log2(np.e) * 1024.0)
SCH_C = -0.058
SCH_B = float((15.0 + SCH_C) * 1024.0)

_NP_MM = {"float16": np.float16, "float32": np.float32}

for _p in ("/opt/trn_rl_repo", "/opt/pypackages"):
    if _p not in sys.path:
        sys.path.append(_p)


def _row_off(ci, s_len):
    # packed column offset of causal row ci: sum_{j<ci} (s_len - 128*j)
    return s_len * ci - CH * (ci * (ci - 1)) // 2


def _build_program(n_heads, s_len, piece_w=PIECE_W, mm_dtype=MM_DTYPE):
    import concourse.bass as bass  # noqa: F401
    import concourse.bacc as bacc
    import concourse.tile as tile
    from concourse import mybir

    f32 = mybir.dt.float32
    i16 = mybir.dt.int16
    mmdt = getattr(mybir.dt, mm_dtype)
    n_chunks = s_len // CH
    n_pairs = (n_heads + 1) // 2
    DP1 = D + 1
    pt_len = _row_off(n_chunks, s_len)

    nc = bacc.Bacc(
        "TRN2",
        target_bir_lowering=False,
        debug=False,
        num_devices=N_CORES,
    )

    qk_d = nc.dram_tensor("qk", [128, n_pairs, 2, s_len], mmdt, kind="ExternalInput").ap()
    v_d = nc.dram_tensor("v", [128, n_heads, n_chunks, DP1], mmdt, kind="ExternalInput").ap()
    o_d = nc.dram_tensor("o", [n_chunks, 128, n_heads * DP1], f32, kind="ExternalOutput").ap()

    # static greedy engine balance; VE cost includes post-op DRAIN
    bal = {"sc": 0.0, "ve": 0.0}
    SC = lambda fd, nseg=1: (230.0 * nseg + fd) / 1.2
    VE = lambda fd: 2.0 * (120.0 + fd) / 0.96 - 266.0

    with tile.TileContext(nc) as tc, ExitStack() as ctx:
        const = ctx.enter_context(tc.tile_pool(name="const", bufs=1))
        sb_pt = ctx.enter_context(tc.tile_pool(name="ptp", bufs=2))
        sb_st = ctx.enter_context(tc.tile_pool(name="stage", bufs=1))
        ps_s = ctx.enter_context(tc.tile_pool(name="pss", bufs=3, space="PSUM"))
        ps_o = ctx.enter_context(tc.tile_pool(name="pso", bufs=2, space="PSUM"))

        wu = const.tile([128, 512], mmdt)  # PE warmup scratch
        nc.gpsimd.memset(wu[:, 0:512], 0.0)
        kq = {p: const.tile([128, s_len], mmdt, name=f"kq{p}") for p in range(n_pairs)}
        qq = {p: const.tile([128, s_len], mmdt, name=f"qq{p}") for p in range(n_pairs)}
        v = const.tile([128, n_heads, n_chunks, DP1], mmdt)

        # PE warmup while input DMAs land
        stw = ps_s.tile([128, 2, piece_w], f32, tag="st")
        for i in range(18):
            bp = 64 * (i % 2)
            nc.tensor.matmul(
                stw[:, i % 2, 0:piece_w],
                wu[bp:bp + 64, 0:CH],
                wu[bp:bp + 64, 0:piece_w],
                start=True, stop=True,
                tile_position=(bp, 0),
            )

        for pair in range(n_pairs):
            nc.sync.dma_start(out=kq[pair], in_=qk_d[:, pair, 1])
            nc.sync.dma_start(out=qq[pair], in_=qk_d[:, pair, 0])
        for hh in range(n_heads):
            nc.sync.dma_start(out=v[:, hh], in_=v_d[:, hh])

        pair_heads = {p: [hh for hh in (2 * p, 2 * p + 1) if hh < n_heads]
                      for p in range(n_pairs)}
        pts = {p: sb_pt.tile([128, 2, pt_len], mmdt, tag="ptfull", name=f"ptp{p}")
               for p in range(n_pairs)}
        stage = sb_st.tile([128, n_chunks, n_heads * DP1], f32, name="ostage")

        def exp_sc(pt_t, src_ap, idx0, nh, dst0, fd):
            nc.scalar.activation(
                pt_t[:, idx0:idx0 + nh, dst0:dst0 + fd],
                src_ap,
                mybir.ActivationFunctionType.Exp,
                scale=float(SCALE),
            )

        def exp_ve(pt_t, src_ap, idx0, nh, dst0, fd):
            nc.vector.tensor_scalar(
                pt_t[:, idx0:idx0 + nh, dst0:dst0 + fd].bitcast(i16),
                src_ap,
                SCH_A,
                SCH_B,
                mybir.AluOpType.mult,
                mybir.AluOpType.add,
            )

        def balance_exp(pt_t, st, nh, dst0, src0, w):
            cb_ve = VE(2 * w)
            cb_sc = 2 * SC(w)
            c_sp = (SC(w), VE(w))
            opts = [
                ("ve", max(bal["sc"], bal["ve"] + cb_ve)),
                ("sc", max(bal["sc"] + cb_sc, bal["ve"])),
                ("sp", max(bal["sc"] + c_sp[0], bal["ve"] + c_sp[1])),
            ]
            choice = min(opts, key=lambda o: o[1])[0]
            if choice == "ve":
                bal["ve"] += cb_ve
                exp_ve(pt_t, st[:, 0:nh, src0:src0 + w], 0, nh, dst0, w)
            elif choice == "sc":
                bal["sc"] += cb_sc
                for idx in range(nh):
                    exp_sc(pt_t, st[:, idx, src0:src0 + w], idx, 1, dst0, w)
            else:
                bal["sc"] += c_sp[0]
                bal["ve"] += c_sp[1]
                exp_sc(pt_t, st[:, 0, src0:src0 + w], 0, 1, dst0, w)
                exp_ve(pt_t, st[:, 1, src0:src0 + w], 1, 1, dst0, w)

        def ph1_row(pair, heads, ci, pt_t):
            sp0 = CH * ci
            span = s_len - sp0
            ro = _row_off(ci, s_len)
            nh = len(heads)
            for poff in range(0, span, piece_w):
                w = min(piece_w, span - poff)
                st = ps_s.tile([128, 2, piece_w], f32, tag="st")
                for idx, hh in enumerate(heads):
                    bp = 64 * (hh % 2)
                    nc.tensor.matmul(
                        st[:, idx, 0:w],
                        kq[pair][bp:bp + 64, sp0:sp0 + CH],
                        qq[pair][bp:bp + 64, sp0 + poff:sp0 + poff + w],
                        start=True, stop=True,
                        tile_position=(bp, 0),
                    )
                if poff == 0:
                    # exact exp on the diagonal block; balance the rest
                    bal["sc"] += SC(CH, nseg=2)
                    exp_sc(pt_t, st[:, 0:nh, 0:CH], 0, nh, ro, CH)
                    if w > CH:
                        balance_exp(pt_t, st, nh, ro + CH, CH, w - CH)
                    for idx in range(nh):
                        nc.gpsimd.affine_select(
                            out=pt_t[:, idx, ro:ro + CH],
                            in_=pt_t[:, idx, ro:ro + CH],
                            compare_op=mybir.AluOpType.is_ge,
                            fill=0.0,
                            base=0,
                            pattern=[[1, CH]],
                            channel_multiplier=-1,
                        )
                else:
                    balance_exp(pt_t, st, nh, ro + poff, 0, w)

        def chain_quad(qt):
            op = ps_o.tile([128, n_heads, DP1], f32, tag="op")
            for pair in range(n_pairs):
                for idx, hh in enumerate(pair_heads[pair]):
                    for ci in range(qt + 1):
                        sl = _row_off(ci, s_len) + CH * (qt - ci)
                        nc.tensor.matmul(
                            op[:, hh, :],
                            pts[pair][:, idx, sl:sl + CH],
                            v[:, hh, ci, :],
                            start=(ci == 0),
                            stop=(ci == qt),
                        )
            fd = n_heads * DP1
            if bal["sc"] + SC(fd) <= bal["ve"] + VE(fd):
                bal["sc"] += SC(fd)
                nc.scalar.activation(
                    stage[:, qt], op[:, :, :],
                    mybir.ActivationFunctionType.Copy,
                )
            else:
                bal["ve"] += VE(fd)
                nc.vector.tensor_copy(stage[:, qt], op[:, :, :])
            nc.sync.dma_start(out=o_d[qt], in_=stage[:, qt])

        LAG = 2
        pending = []
        for ci in range(n_chunks):
            for pair in range(n_pairs):
                ph1_row(pair, pair_heads[pair], ci, pts[pair])
            pending.append(ci)
            while len(pending) > LAG:
                chain_quad(pending.pop(0))
        while pending:
            chain_quad(pending.pop(0))

    nc.compile()
    return nc


_PROGRAM_CACHE = {}


def _get_program(n_heads=HEADS_PER_CORE, s_len=S, piece_w=PIECE_W, mm_dtype=MM_DTYPE):
    key = (n_heads, s_len, piece_w, mm_dtype)
    if key not in _PROGRAM_CACHE:
        _PROGRAM_CACHE[key] = _build_program(n_heads, s_len, piece_w, mm_dtype)
    return _PROGRAM_CACHE[key]


def _pack_core(Qf, Kf, Vf, heads, s_len=S, mm_dtype=MM_DTYPE):
    """Build the per-core input dict. Qf/Kf/Vf: [B*H, S, D] float32."""
    dt_np = _NP_MM[mm_dtype]
    n_heads = len(heads)
    n_pairs = (n_heads + 1) // 2
    n_chunks = s_len // CH
    qk = np.zeros((128, n_pairs, 2, s_len), dt_np)
    v = np.ones((128, n_heads, n_chunks, D + 1), dt_np)
    for i, hf in enumerate(heads):
        pair, side = divmod(i, 2)
        bp = 64 * side
        qk[bp:bp + 64, pair, 0] = Qf[hf].T
        qk[bp:bp + 64, pair, 1] = Kf[hf].T
        v[:, i, :, :D] = Vf[hf].reshape(n_chunks, CH, D).transpose(1, 0, 2)
    return {"qk": qk, "v": v}


def _unpack_core(o_np, s_len=S):
    """o_np: [n_chunks, 128, n_heads*(D+1)] raw -> [n_heads, S, D]."""
    n_chunks = s_len // CH
    o = o_np.reshape(n_chunks, 128, -1, D + 1)
    n_heads = o.shape[2]
    out = o[:, :, :, :D] / o[:, :, :, D:D + 1]
    return out.transpose(2, 0, 1, 3).reshape(n_heads, s_len, D).astype(np.float32)


def kernel(Q, K, V, mask):
    Q = np.asarray(Q, np.float32)
    K = np.asarray(K, np.float32)
    V = np.asarray(V, np.float32)
    mask = np.asarray(mask)

    if not np.array_equal(mask, np.tril(np.ones((S, S), dtype=bool))):
        scores = np.einsum("bhqd,bhkd->bhqk", Q, K) * SCALE
        scores = np.where(mask, scores, -np.inf)
        scores -= scores.max(-1, keepdims=True)
        p = np.exp(scores)
        p /= p.sum(-1, keepdims=True)
        return np.einsum("bhqk,bhkd->bhqd", p, V).astype(np.float32)

    from concourse.bass_utils import run_bass_kernel_spmd

    Qf = Q.reshape(B * H, S, D)
    Kf = K.reshape(B * H, S, D)
    Vf = V.reshape(B * H, S, D)

    nc = _get_program()
    in_maps = [
        _pack_core(Qf, Kf, Vf, list(range(c * HEADS_PER_CORE, (c + 1) * HEADS_PER_CORE)))
        for c in range(N_CORES)
    ]
    res = run_bass_kernel_spmd(nc, in_maps, core_ids=list(range(N_CORES)))
    out = np.empty((B * H, S, D), np.float32)
    for c in range(N_CORES):
        out[c * HEADS_PER_CORE:(c + 1) * HEADS_PER_CORE] = _unpack_core(res.results[c]["o"])
    return out.reshape(B, H, S, D)


# revision 18
# speedup vs baseline: 1.2940x; 1.0287x over previous
"""Causal scaled-dot-product attention on 8 Trainium2 NeuronCores.

Problem: B=2, H=16, S=2048, D=64, fp32, causal mask.
Sharding: batch*heads (32) split 4-per-core across 8 cores; no collectives.

Per-core bass/Tile kernel (v10):
  - mm1: S^T[k,q] = (K^T)^T @ Q^T, fp16, head pair A/B on SBUF partition
    halves -> 64x128 row-tiled matmuls run concurrently (T0/T8).
  - exp: per 512-wide piece from PSUM. The 128-wide diagonal block of
    each row is always exact ScalarE table-exp (self-attention keys
    dominate softmax mass); everything else is load-balanced between
    ScalarE (exact) and DVE via a one-op Schraudolph:
    int16(s*log2e*1024 + (15+c)*1024) bit-cast as fp16 ~= e^s to ~3%;
    the sawtooth cancels through softmax normalization. DVE cost model
    includes its post-op pipeline DRAIN (~dur-266ns).
  - GPSIMD affine_select zeroes the diag upper triangle after exp.
  - Phase 2 (lagging 2 rows): per q-tile all 4 heads accumulate O'|l
    into one PSUM bank ([128, 4, 65], V_aug = [V|1] moving, pt chunks
    stationary); evacuated raw to SBUF/DRAM; host divides by l.
  - PSUM: score pieces 3x[128,2,512] (6 banks) + O' accum 2x1 bank.
"""

import sys
import numpy as np
from contextlib import ExitStack

B, H, S, D = 2, 16, 2048, 64
N_CORES = 8
HEADS_PER_CORE = (B * H) // N_CORES  # 4
CH = 128             # k-chunk (partition tile)
PIECE_W = 512        # S^T piece width (1 PSUM bank per head)
SCALE = 1.0 / np.sqrt(D)
MM_DTYPE = "float16"
SCH_A = float(SCALE * np.log2(np.e) * 1024.0)
SCH_C = -0.058
SCH_B = float((15.0 + SCH_C) * 1024.0)

_NP_MM = {"float16": np.float16, "float32": np.float32}

for _p in ("/opt/trn_rl_repo", "/opt/pypackages"):
    if _p not in sys.path:
        sys.path.append(_p)


def _row_off(ci, s_len):
    # packed column offset of causal row ci: sum_{j<ci} (s_len - 128*j)
    return s_len * ci - CH * (ci * (ci - 1)) // 2


def _build_program(n_heads, s_len, piece_w=PIECE_W, mm_dtype=MM_DTYPE):
    import concourse.bass as bass  # noqa: F401
    import concourse.bacc as bacc
    import concourse.tile as tile
    from concourse import mybir

    f32 = mybir.dt.float32
    i16 = mybir.dt.int16
    mmdt = getattr(mybir.dt, mm_dtype)
    n_chunks = s_len // CH
    n_pairs = (n_heads + 1) // 2
    DP1 = D + 1
    pt_len = _row_off(n_chunks, s_len)

    nc = bacc.Bacc(
        "TRN2",
        target_bir_lowering=False,
        debug=False,
        num_devices=N_CORES,
    )

    qk_d = nc.dram_tensor("qk", [128, n_pairs, 2, s_len], mmdt, kind="ExternalInput").ap()
    v_d = nc.dram_tensor("v", [128, n_heads, n_chunks, DP1], mmdt, kind="ExternalInput").ap()
    o_d = nc.dram_tensor("o", [n_chunks, 128, n_heads * DP1], f32, kind="ExternalOutput").ap()

    # static greedy engine balance; VE cost includes post-op DRAIN
    bal = {"sc": 0.0, "ve": 0.0}
    SC = lambda fd, nseg=1: (230.0 * nseg + fd) / 1.2
    VE = lambda fd: 2.0 * (120.0 + fd) / 0.96 - 266.0

    with tile.TileContext(nc) as tc, ExitStack() as ctx:
        const = ctx.enter_context(tc.tile_pool(name="const", bufs=1))
        sb_pt = ctx.enter_context(tc.tile_pool(name="ptp", bufs=2))
        sb_st = ctx.enter_context(tc.tile_pool(name="stage", bufs=1))
        ps_s = ctx.enter_context(tc.tile_pool(name="pss", bufs=3, space="PSUM"))
        ps_o = ctx.enter_context(tc.tile_pool(name="pso", bufs=2, space="PSUM"))

        wu = const.tile([128, 512], mmdt)  # PE warmup scratch
        nc.gpsimd.memset(wu[:, 0:512], 0.0)
        kq = {p: const.tile([128, s_len], mmdt, name=f"kq{p}") for p in range(n_pairs)}
        qq = {p: const.tile([128, s_len], mmdt, name=f"qq{p}") for p in range(n_pairs)}
        v = const.tile([128, n_heads, n_chunks, DP1], mmdt)

        # PE warmup while input DMAs land
        stw = ps_s.tile([128, 2, piece_w], f32, tag="st")
        for i in range(18):
            bp = 64 * (i % 2)
            nc.tensor.matmul(
                stw[:, i % 2, 0:piece_w],
                wu[bp:bp + 64, 0:CH],
                wu[bp:bp + 64, 0:piece_w],
                start=True, stop=True,
                tile_position=(bp, 0),
            )

        for pair in range(n_pairs):
            nc.sync.dma_start(out=kq[pair], in_=qk_d[:, pair, 1])
            nc.sync.dma_start(out=qq[pair], in_=qk_d[:, pair, 0])
        for hh in range(n_heads):
            nc.sync.dma_start(out=v[:, hh], in_=v_d[:, hh])

        pair_heads = {p: [hh for hh in (2 * p, 2 * p + 1) if hh < n_heads]
                      for p in range(n_pairs)}
        pts = {p: sb_pt.tile([128, 2, pt_len], mmdt, tag="ptfull", name=f"ptp{p}")
               for p in range(n_pairs)}
        stage = sb_st.tile([128, n_chunks, n_heads * DP1], f32, name="ostage")

        def exp_sc(pt_t, src_ap, idx0, nh, dst0, fd):
            nc.scalar.activation(
                pt_t[:, idx0:idx0 + nh, dst0:dst0 + fd],
                src_ap,
                mybir.ActivationFunctionType.Exp,
                scale=float(SCALE),
            )

        def exp_ve(pt_t, src_ap, idx0, nh, dst0, fd):
            nc.vector.tensor_scalar(
                pt_t[:, idx0:idx0 + nh, dst0:dst0 + fd].bitcast(i16),
                src_ap,
                SCH_A,
                SCH_B,
                mybir.AluOpType.mult,
                mybir.AluOpType.add,
            )

        def balance_exp(pt_t, st, nh, dst0, src0, w):
            cb_ve = VE(2 * w)
            cb_sc = 2 * SC(w)
            c_sp = (SC(w), VE(w))
            opts = [
                ("ve", max(bal["sc"], bal["ve"] + cb_ve)),
                ("sc", max(bal["sc"] + cb_sc, bal["ve"])),
                ("sp", max(bal["sc"] + c_sp[0], bal["ve"] + c_sp[1])),
            ]
            choice = min(opts, key=lambda o: o[1])[0]
            if choice == "ve":
                bal["ve"] += cb_ve
                exp_ve(pt_t, st[:, 0:nh, src0:src0 + w], 0, nh, dst0, w)
            elif choice == "sc":
                bal["sc"] += cb_sc
                for idx in range(nh):
                    exp_sc(pt_t, st[:, idx, src0:src0 + w], idx, 1, dst0, w)
            else:
                bal["sc"] += c_sp[0]
                bal["ve"] += c_sp[1]
                exp_sc(pt_t, st[:, 0, src0:src0 + w], 0, 1, dst0, w)
                exp_ve(pt_t, st[:, 1, src0:src0 + w], 1, 1, dst0, w)

        def ph1_row(pair, heads, ci, pt_t):
            sp0 = CH * ci
            span = s_len - sp0
            ro = _row_off(ci, s_len)
            nh = len(heads)
            for poff in range(0, span, piece_w):
                w = min(piece_w, span - poff)
                st = ps_s.tile([128, 2, piece_w], f32, tag="st")
                for idx, hh in enumerate(heads):
                    bp = 64 * (hh % 2)
                    nc.tensor.matmul(
                        st[:, idx, 0:w],
                        kq[pair][bp:bp + 64, sp0:sp0 + CH],
                        qq[pair][bp:bp + 64, sp0 + poff:sp0 + poff + w],
                        start=True, stop=True,
                        tile_position=(bp, 0),
                    )
                if poff == 0:
                    # exact exp on the diagonal block; balance the rest
                    bal["sc"] += SC(CH, nseg=2)
                    exp_sc(pt_t, st[:, 0:nh, 0:CH], 0, nh, ro, CH)
                    if w > CH:
                        balance_exp(pt_t, st, nh, ro + CH, CH, w - CH)
                    for idx in range(nh):
                        nc.gpsimd.affine_select(
                            out=pt_t[:, idx, ro:ro + CH],
                            in_=pt_t[:, idx, ro:ro + CH],
                            compare_op=mybir.AluOpType.is_ge,
                            fill=0.0,
                            base=0,
                            pattern=[[1, CH]],
                            channel_multiplier=-1,
                        )
                else:
                    balance_exp(pt_t, st, nh, ro + poff, 0, w)

        def chain_quad(qt):
            op = ps_o.tile([128, n_heads, DP1], f32, tag="op")
            for pair in range(n_pairs):
                for idx, hh in enumerate(pair_heads[pair]):
                    for ci in range(qt + 1):
                        sl = _row_off(ci, s_len) + CH * (qt - ci)
                        nc.tensor.matmul(
                            op[:, hh, :],
                            pts[pair][:, idx, sl:sl + CH],
                            v[:, hh, ci, :],
                            start=(ci == 0),
                            stop=(ci == qt),
                        )
            fd = n_heads * DP1
            if bal["sc"] + SC(fd) <= bal["ve"] + VE(fd):
                bal["sc"] += SC(fd)
                nc.scalar.activation(
                    stage[:, qt], op[:, :, :],
                    mybir.ActivationFunctionType.Copy,
                )
            else:
                bal["ve"] += VE(fd)
                nc.vector.tensor_copy(stage[:, qt], op[:, :, :])
            nc.sync.dma_start(out=o_d[qt], in_=stage[:, qt])

        LAG = 3
        pending = []
        for ci in range(n_chunks):
            for pair in range(n_pairs):
                ph1_row(pair, pair_heads[pair], ci, pts[pair])
            pending.append(ci)
            while len(pending) > LAG:
                chain_quad(pending.pop(0))
        while pending:
            chain_quad(pending.pop(0))

    nc.compile()
    return nc


_PROGRAM_CACHE = {}


def _get_program(n_heads=HEADS_PER_CORE, s_len=S, piece_w=PIECE_W, mm_dtype=MM_DTYPE):
    key = (n_heads, s_len, piece_w, mm_dtype)
    if key not in _PROGRAM_CACHE:
        _PROGRAM_CACHE[key] = _build_program(n_heads, s_len, piece_w, mm_dtype)
    return _PROGRAM_CACHE[key]


def _pack_core(Qf, Kf, Vf, heads, s_len=S, mm_dtype=MM_DTYPE):
    """Build the per-core input dict. Qf/Kf/Vf: [B*H, S, D] float32."""
    dt_np = _NP_MM[mm_dtype]
    n_heads = len(heads)
    n_pairs = (n_heads + 1) // 2
    n_chunks = s_len // CH
    qk = np.zeros((128, n_pairs, 2, s_len), dt_np)
    v = np.ones((128, n_heads, n_chunks, D + 1), dt_np)
    for i, hf in enumerate(heads):
        pair, side = divmod(i, 2)
        bp = 64 * side
        qk[bp:bp + 64, pair, 0] = Qf[hf].T
        qk[bp:bp + 64, pair, 1] = Kf[hf].T
        v[:, i, :, :D] = Vf[hf].reshape(n_chunks, CH, D).transpose(1, 0, 2)
    return {"qk": qk, "v": v}


def _unpack_core(o_np, s_len=S):
    """o_np: [n_chunks, 128, n_heads*(D+1)] raw -> [n_heads, S, D]."""
    n_chunks = s_len // CH
    o = o_np.reshape(n_chunks, 128, -1, D + 1)
    n_heads = o.shape[2]
    out = o[:, :, :, :D] / o[:, :, :, D:D + 1]
    return out.transpose(2, 0, 1, 3).reshape(n_heads, s_len, D).astype(np.float32)


def kernel(Q, K, V, mask):
    Q = np.asarray(Q, np.float32)
    K = np.asarray(K, np.float32)
    V = np.asarray(V, np.float32)
    mask = np.asarray(mask)

    if not np.array_equal(mask, np.tril(np.ones((S, S), dtype=bool))):
        scores = np.einsum("bhqd,bhkd->bhqk", Q, K) * SCALE
        scores = np.where(mask, scores, -np.inf)
        scores -= scores.max(-1, keepdims=True)
        p = np.exp(scores)
        p /= p.sum(-1, keepdims=True)
        return np.einsum("bhqk,bhkd->bhqd", p, V).astype(np.float32)

    from concourse.bass_utils import run_bass_kernel_spmd

    Qf = Q.reshape(B * H, S, D)
    Kf = K.reshape(B * H, S, D)
    Vf = V.reshape(B * H, S, D)

    nc = _get_program()
    in_maps = [
        _pack_core(Qf, Kf, Vf, list(range(c * HEADS_PER_CORE, (c + 1) * HEADS_PER_CORE)))
        for c in range(N_CORES)
    ]
    res = run_bass_kernel_spmd(nc, in_maps, core_ids=list(range(N_CORES)))
    out = np.empty((B * H, S, D), np.float32)
    for c in range(N_CORES):
        out[c * HEADS_PER_CORE:(c + 1) * HEADS_PER_CORE] = _unpack_core(res.results[c]["o"])
    return out.reshape(B, H, S, D)
